# revision 40
# baseline (speedup 1.0000x reference)
"""GCN autoencoder (2x GCN layer + inner-product decoder) on 8 TRN2 NeuronCores.

Problem (full shapes):
    x [8192, 512] f32, w1 [512, 256] f32, w2 [256, 16] f32,
    edge_weight [262144] f32, row/col [262144] i32
    h1  = relu(segment_sum((x @ w1)[col] * ew, row, 8192))     # [8192, 256]
    z   = segment_sum((h1 @ w2)[col] * ew, row, 8192)          # [8192, 16]
    adj = z @ z.T                                              # [8192, 8192]

Strategy v2 (destination-shard BOTH layers; single resident A-shard):
  Dense A[r, c] = sum of edge_weight over edges (row=r, col=c), bf16.
  Each core owns 1024 destination rows; the A^T column-shard
  ATr[src, own-dest] (16 MiB bf16) is loaded once and stays in SBUF,
  feeding both aggregations.
    P1  support1 = x @ w1            (full, redundant per core)
    L1  h1_c^T  = relu(support1^T @ ATr)              [256, 1024]
    S2  s2_c^T  = w2^T @ h1_c^T                       [16, 1024]
    AG1 AllGather s2^T -> [16, 8192]; PE-transpose to s2 [src, 16]
    L2  z_c^T   = s2^T-chunks (col-tiled x4) @ ATr     [16, 1024]
    AG2 AllGather z^T -> [16, 8192]
    DEC adj_c   = z_c @ z^T  (K=16 matmuls), row-slice written fp32.
"""

import os
import sys

import numpy as np

if "/opt/trn_rl_repo" not in sys.path:
    sys.path.insert(0, "/opt/trn_rl_repo")

import ml_dtypes

import concourse.bass as bass
import concourse.mybir as mybir
import concourse.tile as tile
from concourse import bacc
from concourse.bass_utils import run_bass_kernel_spmd

N = 8192          # nodes
D_IN = 512        # input features
D_H = 256         # hidden features
D_Z = 16          # latent features
NCORES = 8
R = N // NCORES   # 1024 destination rows per core
P = 128

BF = mybir.dt.bfloat16
F32 = mybir.dt.float32

# stash for test harness introspection (exec_time_ns etc.)
LAST_RESULTS = None
_NC_CACHE = None

# col-tiled L2 aggregation (4 concurrent M=16 matmuls); 0 = serial fallback
COLTILE = int(os.environ.get("BASS_L2_COLTILE", "1"))


def _build_kernel(phases=9):
    nc = bacc.Bacc("TRN2", target_bir_lowering=False, debug=False,
                   num_devices=NCORES)

    xT = nc.dram_tensor("xT", [D_IN, N], BF, kind="ExternalInput").ap()
    w1 = nc.dram_tensor("w1", [D_IN, D_H], BF, kind="ExternalInput").ap()
    w2 = nc.dram_tensor("w2", [D_H, D_Z], BF, kind="ExternalInput").ap()
    # A^T row-shard (sources x own-destinations), partition-major:
    # ATr[p, k, r] = A^T[k*128 + p, core*R + r]
    ATr = nc.dram_tensor("ATr", [P, N // P, R], BF, kind="ExternalInput").ap()
    # block-diagonal identity: id16[32j + z, z'] = (z == z'), for row-tiled
    # transposes at partition strips 0/32/64/96
    id16 = nc.dram_tensor("id16", [P, D_Z], BF, kind="ExternalInput").ap()
    # gather matrix summing the 4 col-tile strips: G[32j+z, z'] = (z == z')
    gsum = nc.dram_tensor("gsum", [P, D_Z], F32, kind="ExternalInput").ap()
    adj = nc.dram_tensor("adj", [R, N], F32, kind="ExternalOutput").ap()

    with tile.TileContext(nc) as tc:
        _body(tc, xT, w1, w2, ATr, id16, gsum, adj, phases)
    nc.compile()
    return nc


def _body(tc, xT, w1, w2, ATr, id16, gsum, adj, phases=9):
    nc = tc.nc
    KX = D_IN // P          # 4 k-chunks over input features
    MCH = N // P            # 64 node chunks
    DH_CH = D_H // P        # 2 chunks over hidden features
    XGRP = 4                # node chunks per x DMA (512 KiB)
    AGRP = 4                # ATr k-chunks per DMA (1 MiB)
    RB = R // P             # 8 row blocks per core

    xT_v = xT.rearrange("(k p) n -> p k n", p=P)                  # [128, 4, 8192]
    w1_v = w1.rearrange("(k p) n -> p k n", p=P)                  # [128, 4, 256]
    w2_v = w2.rearrange("(k p) n -> p k n", p=P)                  # [128, 2, 16]

    with (
        tc.tile_pool(name="const", bufs=1) as const,
        tc.tile_pool(name="persist", bufs=1) as persist,
        tc.tile_pool(name="xstream", bufs=3) as xstream,
        tc.tile_pool(name="s1rot", bufs=8) as s1rot,
        tc.tile_pool(name="outbuf", bufs=3) as outbuf,
        tc.tile_pool(name="psum_rot", bufs=2, space="PSUM") as psum_rot,
        tc.tile_pool(name="psum_acc", bufs=1, space="PSUM") as psum_acc,
        tc.tile_pool(name="dram", bufs=1, space="DRAM") as dram,
    ):
        # ---- phase-1 critical constants first (Sync issues DMAs serially
        # at ~640 ns apiece, so issue order sets the PE start time) ----
        w1s = const.tile([P, KX, D_H], BF)
        nc.sync.dma_start(w1s[:], w1_v[:])

        # ---- persistent tiles ----
        asb = persist.tile([P, MCH, R], BF)              # resident A^T shard
        h1T = persist.tile([P, DH_CH, R], BF)            # h1_c^T   [256, 1024]
        s2T_rep = persist.tile([P, NCORES, R], BF)       # s2^T x4 strips
        s2sb = persist.tile([P, MCH, D_Z], BF)           # s2       [8192, 16]
        zT16 = persist.tile([D_Z, R], BF)                # z_c^T    [16, 1024]
        zT_full = persist.tile([D_Z, NCORES, R], BF)     # z^T      [16, 8192]

        # ================= Phase 1: P1 (x @ w1) + L1 (agg into own dests) ==
        ph = [[psum_acc.tile([P, 512], F32, name=f"ph_{dh}_{nn}",
                             tag=f"ph_{dh}_{nn}")
               for nn in range(2)] for dh in range(2)]
        # group m-chunks for x/A streaming: first groups small so the PE
        # starts as soon as possible, then steady XGRP-wide groups. x and A
        # groups interleave on the Sync ring in lockstep with consumption.
        groups = [(0, 1), (1, 1), (2, 2)] + [
            (m0, XGRP) for m0 in range(XGRP, MCH, XGRP)]
        cur = {}
        for (m0, gw) in groups:
            xts = xstream.tile([P, KX, XGRP * P], BF, tag="xts")
            nc.sync.dma_start(
                xts[:, :, :gw * P], xT_v[:, :, m0 * P:(m0 + gw) * P])
            nc.sync.dma_start(asb[:, m0:m0 + gw, :], ATr[:, m0:m0 + gw, :])
            if m0 == 2:
                # constants not needed until after L1, issued once the
                # head of the stream is in flight
                w2s = const.tile([P, DH_CH, D_Z], BF)
                nc.sync.dma_start(w2s[:], w2_v[:])
                ids = const.tile([P, D_Z], BF)
                nc.sync.dma_start(ids[:], id16[:])
                gsum_s = const.tile([P, D_Z], F32)
                nc.sync.dma_start(gsum_s[:], gsum[:])
                # Warm up the collectives engine: the first collective of a
                # NEFF pays a long ncfw cold-start; a dummy AllGather
                # absorbs it under phase 1.
                ccw_in = dram.tile([D_Z, D_Z], BF)
                ccw_out = dram.tile([NCORES, D_Z, D_Z], BF,
                                    addr_space="Shared")
                nc.sync.dma_start(ccw_in[:], id16[0:D_Z, :])
                nc.gpsimd.collective_compute(
                    "AllGather", mybir.AluOpType.bypass,
                    replica_groups=[list(range(NCORES))],
                    ins=[ccw_in[:].opt()], outs=[ccw_out[:].opt()])
            for ml in range(gw):
                m = m0 + ml
                s1p = psum_rot.tile([P, D_H], F32, tag="psrot")
                for k in range(KX):
                    nc.tensor.matmul(
                        s1p[:], lhsT=xts[:, k, ml * P:(ml + 1) * P],
                        rhs=w1s[:, k], start=(k == 0), stop=(k == KX - 1))
                s1 = s1rot.tile([P, D_H], BF, tag="s1")
                nc.vector.tensor_copy(s1[:], s1p[:])
                for dh in range(DH_CH):
                    for nn in range(2):
                        nc.tensor.matmul(
                            ph[dh][nn][:],
                            lhsT=s1[:, dh * P:(dh + 1) * P],
                            rhs=asb[:, m, nn * 512:(nn + 1) * 512],
                            start=(m == 0), stop=(m == MCH - 1))
        for dh in range(DH_CH):
            for nn in range(2):
                nc.vector.tensor_scalar_max(
                    h1T[:, dh, nn * 512:(nn + 1) * 512], ph[dh][nn][:], 0.0)

        if phases < 2:
            return
        # ================= S2: s2_c^T = w2^T @ h1_c^T ======================
        s2T16 = persist.tile([D_Z, R], BF)
        for nn in range(2):
            ps2 = psum_rot.tile([D_Z, 512], F32, tag="psrot")
            for dh in range(DH_CH):
                nc.tensor.matmul(
                    ps2[:], lhsT=w2s[:, dh],
                    rhs=h1T[:, dh, nn * 512:(nn + 1) * 512],
                    start=(dh == 0), stop=(dh == DH_CH - 1))
            nc.vector.tensor_copy(s2T16[:, nn * 512:(nn + 1) * 512], ps2[:])

        if phases < 3:
            return
        # ================= AG1: AllGather s2^T =============================
        cc1_in = dram.tile([D_Z, R], BF)
        cc1_out = dram.tile([NCORES, D_Z, R], BF, addr_space="Shared")
        nc.sync.dma_start(cc1_in[:], s2T16[:])
        nc.gpsimd.collective_compute(
            "AllGather", mybir.AluOpType.bypass,
            replica_groups=[list(range(NCORES))],
            ins=[cc1_in[:].opt()], outs=[cc1_out[:].opt()])
        # replicate the gathered s2^T into partition strips (base partition
        # 96 is not supported by bass, so strip 64 is shared by two banks)
        for sp in (0, 32, 64):
            nc.sync.dma_start(s2T_rep[sp:sp + D_Z, :, :],
                              cc1_out[:].rearrange("g z r -> z g r"))

        if phases < 4:
            return
        # ================= T: PE-transpose s2^T -> s2 [src-part, 16] =======
        # chunk c -> psum bank (c % 4), slot (c // 4); 4 concurrent
        # row-tiled transposes per round
        ptiles = [
            psum_rot.tile([P, 16, D_Z], BF, name="pt0", tag="psrot"),
            psum_rot.tile([P, 16, D_Z], BF, name="pt1", tag="psrot"),
            psum_acc.tile([P, 16, D_Z], BF, name="pt2", tag="ph_0_1"),
            psum_acc.tile([P, 16, D_Z], BF, name="pt3", tag="ph_1_1"),
        ]
        STRIP = (0, 32, 64, 64)
        for c in range(MCH):
            j = c % 4
            sp = STRIP[j]
            nc.tensor.transpose(
                ptiles[j][:, c // 4, :],
                in_=s2T_rep[sp:sp + D_Z, c // RB,
                            (c % RB) * P:(c % RB + 1) * P],
                identity=ids[sp:sp + D_Z, :])
        s2sb_v = s2sb[:].rearrange("p (g j) z -> p j g z", j=4)
        for j in range(4):
            for h in range(2):
                nc.vector.tensor_copy(
                    s2sb_v[:, j, h * 8:(h + 1) * 8, :],
                    ptiles[j][:, h * 8:(h + 1) * 8, :])

        if phases < 5:
            return
        # ================= L2: z_c^T = s2^T @ ATr (col-tiled x4) ===========
        if COLTILE:
            for nb in range(2):
                pz = psum_acc.tile([P, 512], F32, name=f"pz_{nb}",
                                   tag=f"ph_{nb}_0")
                NG = MCH // 4
                for g in range(NG):
                    for j in range(4):
                        k = g * 4 + j
                        nc.tensor.matmul(
                            pz[32 * j:32 * j + D_Z, :],
                            lhsT=s2sb[:, k, :],
                            rhs=asb[:, k, nb * 512:(nb + 1) * 512],
                            start=(g == 0), stop=(g == NG - 1),
                            tile_position=(0, 32 * j))
                # sum the 4 col-tile strips via a PE gather-matmul (DVE cannot
                # read across base partitions)
                pzsb = outbuf.tile([P, 512], F32, tag="rowbuf")
                nc.vector.tensor_copy(pzsb[:], pz[:])
                psg = psum_rot.tile([D_Z, 512], F32, tag="psrot")
                nc.tensor.matmul(psg[:], lhsT=gsum_s[:], rhs=pzsb[:],
                                 start=True, stop=True)
                nc.vector.tensor_copy(
                    zT16[:, nb * 512:(nb + 1) * 512], psg[:])
        else:
            for nb in range(2):
                pz = psum_acc.tile([D_Z, 512], F32, name=f"pz_{nb}",
                                   tag=f"ph_{nb}_0")
                for k in range(MCH):
                    nc.tensor.matmul(
                        pz[:], lhsT=s2sb[:, k, :],
                        rhs=asb[:, k, nb * 512:(nb + 1) * 512],
                        start=(k == 0), stop=(k == MCH - 1))
                nc.vector.tensor_copy(zT16[:, nb * 512:(nb + 1) * 512], pz[:])

        if phases < 6:
            return
        # ================= AG2: AllGather z^T ==============================
        cc2_in = dram.tile([D_Z, R], BF)
        cc2_out = dram.tile([NCORES, D_Z, R], BF, addr_space="Shared")
        nc.sync.dma_start(cc2_in[:], zT16[:])
        nc.gpsimd.collective_compute(
            "AllGather", mybir.AluOpType.bypass,
            replica_groups=[list(range(NCORES))],
            ins=[cc2_in[:].opt()], outs=[cc2_out[:].opt()])
        nc.sync.dma_start(zT_full[:], cc2_out[:].rearrange("g z r -> z g r"))

        if phases < 7:
            return
        # ================= DEC: adj_c = z_c @ z^T ==========================
        # Two-bank PSUM tiles (2 matmuls each), one wide PSUM->SBUF copy per
        # tile, output DMAs alternating between the Sync and Scalar rings.
        zT_full_f = zT_full[:].rearrange("z g r -> z (g r)")
        OWID = 1024  # output DMA chunk width (512 KiB per transfer)
        for mb in range(RB):
            for og in range(N // OWID):
                u = mb * (N // OWID) + og
                rowbuf = outbuf.tile([P, OWID], F32, tag="rowbuf")
                po = psum_acc.tile(
                    [P, OWID], F32, name=f"po_{mb}_{og}",
                    tag=f"ph_{u % 2}_0")
                for ol in range(OWID // 512):
                    nb = og * (OWID // 512) + ol
                    nc.tensor.matmul(
                        po[:, ol * 512:(ol + 1) * 512],
                        lhsT=zT16[:, mb * P:(mb + 1) * P],
                        rhs=zT_full_f[:, nb * 512:(nb + 1) * 512],
                        start=True, stop=True)
                # split wide PSUM drains between DVE and ACT (3:1)
                if u % 4 == 3:
                    nc.scalar.copy(rowbuf[:], po[:])
                else:
                    nc.vector.tensor_copy(rowbuf[:], po[:])
                dst = adj[mb * P:(mb + 1) * P, og * OWID:(og + 1) * OWID]
                if u % 2 == 0:
                    nc.sync.dma_start(dst, rowbuf[:])
                else:
                    nc.scalar.dma_start(dst, rowbuf[:])


def _get_nc():
    global _NC_CACHE
    phases = int(os.environ.get("BASS_KERNEL_PHASES", "9"))
    if _NC_CACHE is None or _NC_CACHE[0] != phases:
        _NC_CACHE = (phases, _build_kernel(phases))
    return _NC_CACHE[1]


def kernel(x, w1, w2, edge_weight, row, col):
    global LAST_RESULTS
    x = np.asarray(x, dtype=np.float32)
    w1 = np.asarray(w1, dtype=np.float32)
    w2 = np.asarray(w2, dtype=np.float32)
    edge_weight = np.asarray(edge_weight, dtype=np.float32)
    row = np.asarray(row, dtype=np.int64)
    col = np.asarray(col, dtype=np.int64)

    bf16 = ml_dtypes.bfloat16

    # Dense A^T: AT[c, r] = sum of edge_weight over edges with (row=r, col=c)
    # i.e. AT[source, dest]
    AT_dense = np.zeros((N, N), dtype=np.float32)
    np.add.at(AT_dense, (col, row), edge_weight)
    AT_bf = AT_dense.astype(bf16)

    xT_bf = np.ascontiguousarray(x.T).astype(bf16)
    w1_bf = w1.astype(bf16)
    w2_bf = w2.astype(bf16)
    id16_bf = np.zeros((P, D_Z), dtype=bf16)
    for j in range(4):
        id16_bf[32 * j:32 * j + D_Z, :] = np.eye(D_Z, dtype=bf16)
    gsum_np = np.zeros((P, D_Z), dtype=np.float32)
    for j in range(4):
        gsum_np[32 * j:32 * j + D_Z, :] = np.eye(D_Z, dtype=np.float32)

    in_maps = []
    for c in range(NCORES):
        # row shard: [src, own-dest] -> partition-major [128, 64, R]
        atr = AT_bf[:, c * R:(c + 1) * R]                 # [8192, 1024]
        atr = np.ascontiguousarray(
            atr.reshape(N // P, P, R).transpose(1, 0, 2))  # [128, 64, 1024]
        in_maps.append({
            "xT": xT_bf,
            "w1": w1_bf,
            "w2": w2_bf,
            "ATr": atr,
            "id16": id16_bf,
            "gsum": gsum_np,
        })

    nc = _get_nc()
    print("kernel: launching on 8 cores", flush=True)
    res = run_bass_kernel_spmd(nc, in_maps, core_ids=list(range(NCORES)))
    print("kernel: run complete", flush=True)
    LAST_RESULTS = res
    adj = np.concatenate([res.results[c]["adj"] for c in range(NCORES)], axis=0)
    return np.ascontiguousarray(adj.astype(np.float32))
